# revision 25
# baseline (speedup 1.0000x reference)
"""Trainium2 Bass kernel for nn_Attention_89137751261457.

Full attention with 3D RoPE + QK RMSNorm, B=1, N=4096, C=2048, 16 heads,
head_dim=128. Sharded tensor-parallel by head across 8 NeuronCores
(2 heads per core); the output projection is computed per-core on the
head slice and the 8 partial outputs are summed on the host.

Numerics: fp8 DoubleRow (0.5 cyc/row, K=256) everywhere it is accuracy-free:
 - qkv matmul: 3-term hi/lo fp8 split of x and W (x*8, W_qk*256, W_v*4 host
   scaling keeps hi/lo parts in fp8 normal range; the q/k scale cancels in
   RMS-norm, v's net *32 is divided out on the host).
 - attention AV+den: exp outputs fp8 (exp(s*scale - 2), softmax-shift
   invariant) for 24 of 32 m-chunks; v is kept exact via an fp8 hi/lo pair.
   The remaining 8 chunks stay f16 to bound the fp8 quantization error.
 - projection: 3-term hi/lo fp8 of o (carries v's *32) and W_p (*256);
   partial outputs carry *8192, divided out on the host.
Everything else stored f16 instead of bf16 (same cost, 8x less rounding).

Self-contained: hardcodes all shapes; imports only numpy/ml_dtypes/concourse.
"""

import numpy as np
import ml_dtypes

import concourse.bass as bass  # noqa: F401
import concourse.bacc as bacc
import concourse.mybir as mybir
import concourse.tile as tile  # noqa: F401
from concourse.bass_utils import run_bass_kernel_spmd
from concourse.tile import TileContext

BF16 = ml_dtypes.bfloat16
F8 = ml_dtypes.float8_e4m3
F8L = ml_dtypes.float8_e5m2
F16 = np.float16

NUM_HEADS = 16
DIM = 2048
N = 4096
HD = 128          # head dim
P = 128           # partitions
NCORES = 8
HPC = 2           # heads per core
RMS_EPS = 1e-6
ROPE_THETA = 10000.0
SCALE = float(HD) ** -0.5

KO = DIM // P     # 16 contraction chunks
NB = N // 512     # 8 n-chunks of 512 (phase 1)
QB = N // 512     # 8 q-blocks of 512 (phase 2)
MI = N // P       # 32 m-chunks of 128
NPAIR = MI // 2   # 16 m-chunk pairs (phase 2)
F8PAIR = 12       # pairs 0..11 use fp8 atb (24 chunks), rest f16
COLS = 3 * HPC    # 6 col chunks of 128 (q0,k0,v0,q1,k1,v1)

SX = 8.0          # x scale (hi/lo fp8 range)
SWQK = 256.0      # W_q/W_k scale (cancelled by rmsnorm)
SWV = 4.0         # W_v scale (v carries SX*SWV = 32)
SWP = 256.0       # W_proj scale (out carries 32*256, host divides)
SQK = SX * SWQK   # q/k psum scale (2048)
OUT_DIV = SX * SWV * SWP  # 8192

_CACHE = {}


# --------------------------------------------------------------------------
# host-side helpers
# --------------------------------------------------------------------------

def _rope_ext_tables(T, H, W):
    """cos_ext, sin_ext of shape (HD, N): extended interleaved RoPE tables.

    q_rot[d, n] = cos_ext[d, n] * q[d, n] + sin_ext[d, n] * q[pair(d), n]
    where pair(2i) = 2i+1, pair(2i+1) = 2i.
    """
    dt_ = HD // 2
    dh = HD // 4
    dw = HD - dt_ - dh

    def ax(L, d):
        inv = 1.0 / (ROPE_THETA ** (np.arange(0, d, 2, dtype=np.float32) / d))
        return np.arange(L, dtype=np.float32)[:, None] * inv

    ft = np.broadcast_to(ax(T, dt_)[:, None, None, :], (T, H, W, dt_ // 2))
    fh = np.broadcast_to(ax(H, dh)[None, :, None, :], (T, H, W, dh // 2))
    fw = np.broadcast_to(ax(W, dw)[None, None, :, :], (T, H, W, dw // 2))
    f = np.concatenate([ft, fh, fw], axis=-1).reshape(T * H * W, HD // 2)
    cos = np.cos(f).astype(np.float32).T   # (64, N)
    sin = np.sin(f).astype(np.float32).T   # (64, N)
    n_tot = T * H * W
    cos_ext = np.repeat(cos, 2, axis=0)    # (128, N)
    sin_ext = np.empty((HD, n_tot), np.float32)
    sin_ext[0::2] = -sin
    sin_ext[1::2] = sin
    return cos_ext, sin_ext


def _pair_swap(v):
    """Swap adjacent pairs of a (128,) vector."""
    return v.reshape(HD // 2, 2)[:, ::-1].reshape(HD)


def _split8(x, lo_dt=F8):
    """hi/lo fp8 decomposition."""
    hi = np.asarray(x, np.float32).astype(F8)
    lo = (np.asarray(x, np.float32) - hi.astype(np.float32)).astype(lo_dt)
    return hi, lo


# --------------------------------------------------------------------------
# device program
# --------------------------------------------------------------------------

def _build_nc(qkv_bufs=4, sq_bufs=1, qs_bufs=2, tr_bufs=1,
              s_bufs=2, o_bufs=2, den_bufs=1, out_bufs=1,
              xt_bufs=2, wk_bufs=3, at_bufs=6, cs_bufs=2,
              unify_tables=True, LAGN=2):
    nc = bacc.Bacc("TRN2", target_bir_lowering=False, debug=False)
    f32 = mybir.dt.float32
    f16 = mybir.dt.float16
    f8 = mybir.dt.float8e4
    f8l = mybir.dt.float8e5
    DR = mybir.MatmulPerfMode.DoubleRow

    xh = nc.dram_tensor("xh", [DIM, N], f8, kind="ExternalInput")
    xl = nc.dram_tensor("xl", [DIM, N], f8, kind="ExternalInput")
    wh = nc.dram_tensor("wh", [DIM, COLS * P], f8, kind="ExternalInput")
    wl = nc.dram_tensor("wl", [DIM, COLS * P], f8l, kind="ExternalInput")
    bqkv = nc.dram_tensor("bqkv", [P, COLS], f32, kind="ExternalInput")
    bqkv2 = nc.dram_tensor("bqkv2", [P, COLS], f32, kind="ExternalInput")
    wph = nc.dram_tensor("wph", [P, HPC, DIM], f8, kind="ExternalInput")
    wpl = nc.dram_tensor("wpl", [P, HPC, DIM], f8, kind="ExternalInput")
    cosq = nc.dram_tensor("cosq", [P, N], f16, kind="ExternalInput")
    sinq = nc.dram_tensor("sinq", [P, N], f16, kind="ExternalInput")
    cosk = nc.dram_tensor("cosk", [P, N], f16, kind="ExternalInput")
    sink = nc.dram_tensor("sink", [P, N], f16, kind="ExternalInput")
    ones16 = nc.dram_tensor("ones16", [P, 1], f16, kind="ExternalInput")
    ones8 = nc.dram_tensor("ones8", [P, 2, 16], f8, kind="ExternalInput")
    perm = nc.dram_tensor("perm", [P, P], f16, kind="ExternalInput")
    ident = nc.dram_tensor("ident", [P, P], f16, kind="ExternalInput")
    epsc = nc.dram_tensor("epsc", [2, 1], f32, kind="ExternalInput")
    neg2 = nc.dram_tensor("neg2", [P, 1], f32, kind="ExternalInput")
    out_p = nc.dram_tensor("out_p", [N, DIM], f16, kind="ExternalOutput")

    Exp = mybir.ActivationFunctionType.Exp
    Square = mybir.ActivationFunctionType.Square
    Log = mybir.ActivationFunctionType.Ln
    Ident = mybir.ActivationFunctionType.Identity
    MULT = mybir.AluOpType.mult
    ADD = mybir.AluOpType.add
    SUB = mybir.AluOpType.subtract

    xh_v = xh[:].rearrange("(ko p) n -> p ko n", p=P)    # (128, 16, 4096)
    xl_v = xl[:].rearrange("(ko p) n -> p ko n", p=P)
    wh_v = wh[:].rearrange("(ko p) c -> p ko c", p=P)    # (128, 16, 768)
    wl_v = wl[:].rearrange("(ko p) c -> p ko c", p=P)

    cs_dram = {("q", 0): cosq, ("q", 1): sinq,
               ("k", 0): cosk, ("k", 1): sink}

    with TileContext(nc) as tc:
      with tc.tile_pool(name="persist", bufs=1) as pers:
        # persistent sbuf tensors; weights split into ko-chunks so the first
        # qkv matmuls start after a fraction of the weight DMA.
        wh_sb = [pers.tile([P, 4, COLS * P], f8, tag=f"wh{c}",
                           name=f"wh_sb{c}") for c in range(4)]
        wl_sb = [pers.tile([P, 4, COLS * P], f8l, tag=f"wl{c}",
                           name=f"wl_sb{c}") for c in range(4)]
        nc.sync.dma_start(wh_sb[0][:], wh_v[:, 0:4, :])
        nc.sync.dma_start(wl_sb[0][:], wl_v[:, 0:4, :])
        bq_sb = pers.tile([P, COLS], f32, tag="bq_sb")
        nc.sync.dma_start(bq_sb[:], bqkv[:])
        bq2_sb = pers.tile([P, COLS], f32, tag="bq2_sb")
        nc.sync.dma_start(bq2_sb[:], bqkv2[:])
        ones16_sb = pers.tile([P, 1], f16, tag="ones16_sb")
        nc.sync.dma_start(ones16_sb[:], ones16[:])
        # [P, 2, 16] so the DoubleRow pair stride is 16B-aligned; the den
        # matmul slices [:, :, 0:1]
        ones8_sb = pers.tile([P, 2, 16], f8, tag="ones8_sb")
        nc.sync.dma_start(ones8_sb[:], ones8[:])
        perm_sb = pers.tile([P, P], f16, tag="perm_sb")
        nc.sync.dma_start(perm_sb[:], perm[:])
        id_sb = pers.tile([P, P], f16, tag="id_sb")
        nc.sync.dma_start(id_sb[:], ident[:])
        eps_sb = pers.tile([2, 1], f32, tag="eps_sb")
        nc.sync.dma_start(eps_sb[:], epsc[:])
        neg2_sb = pers.tile([P, 1], f32, tag="neg2_sb")
        nc.sync.dma_start(neg2_sb[:], neg2[:])
        wph_sb = pers.tile([P, HPC, DIM], f8, tag="wph_sb")
        wpl_sb = pers.tile([P, HPC, DIM], f8, tag="wpl_sb")

        qT = [pers.tile([P, N], f16, tag=f"qT{h}", name=f"qT{h}")
              for h in range(HPC)]
        kT = [pers.tile([P, N], f16, tag=f"kT{h}", name=f"kT{h}")
              for h in range(HPC)]
        # v: fp8 hi/lo pairs for m-chunks 0..23, f16 for chunks 24..31
        vh_sb = [pers.tile([P, 2 * F8PAIR, HD], f8, tag=f"vh{h}",
                           name=f"vh{h}") for h in range(HPC)]
        vl_sb = [pers.tile([P, 2 * F8PAIR, HD], f8, tag=f"vl{h}",
                           name=f"vl{h}") for h in range(HPC)]
        vn16 = [pers.tile([P, MI - 2 * F8PAIR, HD], f16, tag=f"vn16{h}",
                          name=f"vn16{h}") for h in range(HPC)]
        # normalized attention output, hi/lo fp8, (d, qc, head, q)
        oh_sb = pers.tile([P, MI, HPC, HD], f8, tag="oh_sb")
        ol_sb = pers.tile([P, MI, HPC, HD], f8l, tag="ol_sb")
        # mean-square / rms-scale stash: one [1, N] tile per (head, q|k),
        # all at partition 0.  After the per-nb Ln/Exp chain the tile holds
        # the rms scale (the Exp writes back over the mean-square values).
        ms_t = {(hh, qk): pers.tile([1, N], f16, tag=f"ms{hh}{qk}",
                                    name=f"ms_{hh}_{qk}")
                for hh in range(HPC) for qk in range(2)}

        # ------------------------------------------------------------------
        # emitters (generators yield at interleave points)
        # ------------------------------------------------------------------

        def p1_gen(hh, nb, p1, pcs, wk, ps_qkv, ps_sq, ps_qs, ps_tr):
            """qkv (3-term fp8 DoubleRow) + rope + v-transpose/split for one
            head and one 512-token chunk.  Yields after each col-block."""
            nsl = slice(nb * 512, (nb + 1) * 512)
            xh_t = [p1.tile([P, 4, 512], f8, tag=f"xh{c}",
                            name=f"xh_{hh}_{nb}_{c}") for c in range(4)]
            xl_t = [p1.tile([P, 4, 512], f8, tag=f"xl{c}", bufs=1,
                            name=f"xl_{hh}_{nb}_{c}") for c in range(4)]
            for c in range(4):
                nc.sync.dma_start(xh_t[c][:], xh_v[:, 4 * c:4 * c + 4, nsl])
                if hh == 0 and nb == 0 and c < 3:
                    # interleave remaining weight chunks with the first x
                    # chunks so neither stream starves
                    nc.sync.dma_start(wh_sb[c + 1][:],
                                      wh_v[:, 4 * c + 4:4 * c + 8, :])
                nc.sync.dma_start(xl_t[c][:], xl_v[:, 4 * c:4 * c + 4, nsl])
                if hh == 0 and nb == 0 and c < 3:
                    nc.sync.dma_start(wl_sb[c + 1][:],
                                      wl_v[:, 4 * c + 4:4 * c + 8, :])
            cs_t = {}
            for key, dram in cs_dram.items():
                t = pcs.tile([P, 512], f16, tag=f"cs_{key[0]}_{key[1]}",
                             name=f"cs_{key[0]}_{key[1]}_{hh}_{nb}")
                nc.sync.dma_start(t[:], dram[:, nsl])
                cs_t[key] = t
            if hh == 0 and nb == 2:
                nc.sync.dma_start(wph_sb[:], wph[:])
                nc.sync.dma_start(wpl_sb[:], wpl[:])
            urope = {}
            for t3 in range(3):   # 0=q 1=k 2=v
                col = hh * 3 + t3
                csl = slice(col * P, (col + 1) * P)
                pq = ps_qkv.tile([P, 512], f32, tag="pq")
                # 3-term fp8: wh*xh + wh*xl + wl*xh, DoubleRow over ko
                # pairs (contraction 256 per instruction)
                terms = ((wh_sb, xh_t), (wh_sb, xl_t), (wl_sb, xh_t))
                nt = len(terms) * 8
                it = 0
                for (wsb, xt_) in terms:
                    for j in range(8):
                        c, jj = j // 2, 2 * (j % 2)
                        nc.tensor.matmul(
                            pq[:], lhsT=wsb[c][:, jj:jj + 2, csl],
                            rhs=xt_[c][:, jj:jj + 2, :],
                            start=(it == 0), stop=(it == nt - 1),
                            perf_mode=DR)
                        it += 1
                bias_col = bq_sb[:, col:col + 1]
                if t3 == 2:   # v: bias-add copy, then transpose+split
                    sv = wk.tile([P, 512], f16, tag="sv", bufs=2)
                    nc.scalar.activation(sv[:], pq[:], Ident, bias=bias_col)
                    ptr = ps_tr.tile([P, 512], f16, tag="ptr")
                    for j in range(4):
                        nc.tensor.transpose(ptr[:, j * P:(j + 1) * P],
                                            sv[:, j * P:(j + 1) * P],
                                            id_sb[:])
                    if nb < 2 * F8PAIR // 4:   # fp8 m-chunks
                        vhd = vh_sb[hh][:, nb * 4:(nb + 1) * 4, :] \
                            .rearrange("p a b -> p (a b)")
                        vld = vl_sb[hh][:, nb * 4:(nb + 1) * 4, :] \
                            .rearrange("p a b -> p (a b)")
                        nc.vector.tensor_copy(vhd, ptr[:])
                        nc.vector.tensor_tensor(vld, ptr[:], vhd, SUB)
                    else:                      # f16 m-chunks
                        nb0 = nb - 2 * F8PAIR // 4
                        nc.vector.tensor_copy(
                            vn16[hh][:, nb0 * 4:(nb0 + 1) * 4, :]
                            .rearrange("p a b -> p (a b)"), ptr[:])
                else:        # q or k: square for rms, biased copy, rope
                    tg = "q" if t3 == 0 else "k"
                    # Square descales via its scale param so the f16
                    # output stays in range: ((pq/SQK) + b_true)^2
                    sq = wk.tile([P, 512], f16, tag="sq", bufs=2)
                    nc.scalar.activation(sq[:], pq[:], Square,
                                         bias=bq2_sb[:, col:col + 1],
                                         scale=1.0 / SQK)
                    psq = ps_sq.tile([1, 512], f32, tag="psq")
                    nc.tensor.matmul(psq[:], lhsT=ones16_sb[:], rhs=sq[:],
                                     start=True, stop=True)
                    nc.vector.tensor_copy(ms_t[(hh, t3)][0:1, nsl], psq[:])
                    qb16 = wk.tile([P, 512], f16, tag="qb16", bufs=2)
                    nc.scalar.activation(qb16[:], pq[:], Ident, bias=bias_col)
                    pqs = ps_qs.tile([P, 512], f32, tag="pqs")
                    nc.tensor.matmul(pqs[:], lhsT=perm_sb[:], rhs=qb16[:],
                                     start=True, stop=True)
                    ta = wk.tile([P, 512], f16, tag="ta", bufs=2)
                    nc.vector.tensor_tensor(ta[:], qb16[:], cs_t[(tg, 0)][:],
                                            MULT)
                    tb = wk.tile([P, 512], f16, tag="tb", bufs=2)
                    nc.vector.tensor_tensor(tb[:], pqs[:], cs_t[(tg, 1)][:],
                                            MULT)
                    ur = wk.tile([P, 512], f16, tag=f"u{t3}", bufs=2,
                                 name=f"u_{hh}_{nb}_{t3}")
                    nc.vector.tensor_tensor(ur[:], ta[:], tb[:], ADD)
                    urope[t3] = ur
                yield
            for qk in range(2):
                mst = ms_t[(hh, qk)]
                lmsh = wk.tile([1, 512], f16, tag="lmsh", bufs=2,
                               name=f"lmsh_{hh}_{nb}_{qk}")
                nc.scalar.activation(lmsh[:], mst[0:1, nsl], Log,
                                     bias=eps_sb[0:1, :], scale=1.0 / HD)
                nc.scalar.activation(mst[0:1, nsl], lmsh[:], Exp, scale=-0.5)
                rsb = wk.tile([P, 512], f16, tag="rsb", bufs=2,
                              name=f"rsb_{hh}_{nb}_{qk}")
                nc.gpsimd.partition_broadcast(rsb[:], mst[0:1, nsl])
                dst = qT[hh] if qk == 0 else kT[hh]
                nc.vector.tensor_tensor(dst[:, nsl], urope[qk][:], rsb[:],
                                        MULT)
            yield

        def p2_gen(h, qb, p2, p2s, ps_s, ps_o, ps_den, group=4):
            """Attention for one head and one 512-query block; softmax-exp
            in fp8 for pairs < F8PAIR (DoubleRow AV+den), f16 for the rest.
            Yields after every `group` m-chunk pairs."""
            qsl = slice(qb * 512, (qb + 1) * 512)
            po = ps_o.tile([P, 512], f32, tag="po")
            pden = ps_den.tile([1, 512], f32, tag="pden")
            # software-pipelined: av/den trail the score matmuls so the PE
            # never waits on the exp latency
            atbs = {}

            def _avden(i):
                first = (i == 0)
                last = (i == NPAIR - 1)
                a = atbs[i]
                if i < F8PAIR:
                    nc.tensor.matmul(
                        po[:], lhsT=vh_sb[h][:, 2 * i:2 * i + 2, :],
                        rhs=a[:], start=first, stop=False, perf_mode=DR)
                    nc.tensor.matmul(
                        po[:], lhsT=vl_sb[h][:, 2 * i:2 * i + 2, :],
                        rhs=a[:], start=False, stop=False, perf_mode=DR)
                    nc.tensor.matmul(
                        pden[:], lhsT=ones8_sb[:, :, 0:1], rhs=a[:],
                        start=first, stop=False, perf_mode=DR)
                else:
                    i0 = 2 * (i - F8PAIR)
                    for d in range(2):
                        nc.tensor.matmul(
                            po[:], lhsT=vn16[h][:, i0 + d, :], rhs=a[:, d, :],
                            start=False, stop=(last and d == 1))
                        nc.tensor.matmul(
                            pden[:], lhsT=ones16_sb[:], rhs=a[:, d, :],
                            start=False, stop=(last and d == 1))
                del atbs[i]

            for i in range(NPAIR):
                ps = ps_s.tile([P, 1024], f32, tag="ps")
                nc.tensor.matmul(
                    ps[:, 0:512], lhsT=kT[h][:, 2 * i * P:(2 * i + 1) * P],
                    rhs=qT[h][:, qsl], start=True, stop=True)
                nc.tensor.matmul(
                    ps[:, 512:1024],
                    lhsT=kT[h][:, (2 * i + 1) * P:(2 * i + 2) * P],
                    rhs=qT[h][:, qsl], start=True, stop=True)
                if i < F8PAIR:
                    atb = p2.tile([P, 2, 512], f8, tag="atb8", bufs=4)
                else:
                    atb = p2.tile([P, 2, 512], f16, tag="atb16", bufs=3)
                # exp(s*SCALE - 2): shift keeps fp8 in range; exact softmax
                # invariance (numerator and den share it)
                nc.scalar.activation(atb[:].rearrange("p a b -> p (a b)"),
                                     ps[:], Exp, bias=neg2_sb[:], scale=SCALE)
                atbs[i] = atb
                if i >= LAGN:
                    _avden(i - LAGN)
                if (i + 1) % group == 0 and i != NPAIR - 1:
                    yield
            for i in range(NPAIR - LAGN, NPAIR):
                _avden(i)
            rden = p2s.tile([1, 512], f16, tag="rden")
            with nc.allow_low_precision(reason="1/den in f16: 0.05% on a "
                                        "smooth per-token normalizer"):
                nc.vector.reciprocal(rden[:], pden[:])
            rdb = p2s.tile([P, 512], f16, tag="rdb")
            nc.gpsimd.partition_broadcast(rdb[:], rden[:])
            # normalize + hi/lo fp8 split for the projection
            t16 = p2s.tile([P, 512], f16, tag="t16")
            nc.vector.tensor_tensor(t16[:], po[:], rdb[:], MULT)
            ohd = oh_sb[:, qb * 4:(qb + 1) * 4, h, :]
            old = ol_sb[:, qb * 4:(qb + 1) * 4, h, :]
            t16v = t16[:].rearrange("p (a b) -> p a b", a=4)
            nc.vector.tensor_copy(ohd, t16v)
            nc.vector.tensor_tensor(old, t16v, ohd, SUB)
            yield

        def proj_qb(qb, p3, ps_out):
            """Output projection for one 512-query block (both heads).
            Yields after each 128-token chunk so it can interleave with the
            next block's attention (fills PE gaps in the exp chain)."""
            for qc in range(qb * 4, qb * 4 + 4):
                osb = p3.tile([P, DIM], f16, tag="osb", bufs=2)
                for ob in range(DIM // 512):
                    obsl = slice(ob * 512, (ob + 1) * 512)
                    pout = ps_out.tile([P, 512], f32, tag="pout")
                    # 3-term fp8: oh*wph + ol*wph + oh*wpl, DoubleRow pairs
                    # the two heads (K=256)
                    nc.tensor.matmul(pout[:], lhsT=oh_sb[:, qc, :, :],
                                     rhs=wph_sb[:, :, obsl],
                                     start=True, stop=False, perf_mode=DR)
                    nc.tensor.matmul(pout[:], lhsT=ol_sb[:, qc, :, :],
                                     rhs=wph_sb[:, :, obsl],
                                     start=False, stop=False, perf_mode=DR)
                    nc.tensor.matmul(pout[:], lhsT=oh_sb[:, qc, :, :],
                                     rhs=wpl_sb[:, :, obsl],
                                     start=False, stop=True, perf_mode=DR)
                    nc.vector.tensor_copy(osb[:, obsl], pout[:])
                    if ob % 2 == 1:  # DMA each half as it lands
                        hsl = slice((ob - 1) * 512, (ob + 1) * 512)
                        nc.sync.dma_start(out_p[qc * P:(qc + 1) * P, hsl],
                                          osb[:, hsl])
                yield

        # ------------------------------------------------------------------
        # three pipelined segments (PSUM pools sized to 8 banks each):
        #   A: phase1 head0            (PE-heavy, Act/DVE warm-up)
        #   B: phase1 head1 interleaved with attention head0 (fills the
        #      Act-idle of phase1 with exps; x/cos/sin re-streamed)
        #   C: attention head1 + projection
        # ------------------------------------------------------------------
        with tc.tile_pool(name="p1_sb", bufs=xt_bufs) as p1, \
             tc.tile_pool(name="p1_cs", bufs=cs_bufs) as pcs, \
             tc.tile_pool(name="p1_wk", bufs=wk_bufs) as wk, \
             tc.tile_pool(name="p2_sb", bufs=at_bufs) as p2, \
             tc.tile_pool(name="p2_sm", bufs=2) as p2s, \
             tc.tile_pool(name="p3_sb", bufs=2) as p3:
            with tc.tile_pool(name="psA_qkv", bufs=qkv_bufs, space="PSUM") as a_qkv, \
                 tc.tile_pool(name="psA_sq", bufs=sq_bufs, space="PSUM") as a_sq, \
                 tc.tile_pool(name="psA_qs", bufs=qs_bufs, space="PSUM") as a_qs, \
                 tc.tile_pool(name="psA_tr", bufs=tr_bufs, space="PSUM") as a_tr:
                for nb in range(NB):
                    for _ in p1_gen(0, nb, p1, pcs, wk, a_qkv, a_sq, a_qs,
                                    a_tr):
                        pass
            with tc.tile_pool(name="psB_qkv", bufs=1, space="PSUM") as b_qkv, \
                 tc.tile_pool(name="psB_sq", bufs=1, space="PSUM") as b_sq, \
                 tc.tile_pool(name="psB_qs", bufs=1, space="PSUM") as b_qs, \
                 tc.tile_pool(name="psB_tr", bufs=1, space="PSUM") as b_tr, \
                 tc.tile_pool(name="psB_s", bufs=1, space="PSUM") as b_s, \
                 tc.tile_pool(name="psB_o", bufs=1, space="PSUM") as b_o, \
                 tc.tile_pool(name="psB_den", bufs=1, space="PSUM") as b_den:
                for s in range(NB):
                    g1 = p1_gen(1, s, p1, pcs, wk, b_qkv, b_sq, b_qs, b_tr)
                    g2 = p2_gen(0, s, p2, p2s, b_s, b_o, b_den, group=4)
                    alive = [g1, g2]
                    while alive:
                        for g in list(alive):
                            if next(g, "end") == "end":
                                alive.remove(g)
            with tc.tile_pool(name="psC_s", bufs=s_bufs, space="PSUM") as c_s, \
                 tc.tile_pool(name="psC_o", bufs=o_bufs, space="PSUM") as c_o, \
                 tc.tile_pool(name="psC_den", bufs=den_bufs, space="PSUM") as c_den, \
                 tc.tile_pool(name="psC_out", bufs=out_bufs, space="PSUM") as c_out:
                gp = None   # previous block's projection, interleaved
                for qb in range(QB):
                    g2 = p2_gen(1, qb, p2, p2s, c_s, c_o, c_den, group=4)
                    alive = [g for g in (g2, gp) if g is not None]
                    while alive:
                        for g in list(alive):
                            if next(g, "end") == "end":
                                alive.remove(g)
                    gp = proj_qb(qb, p3, c_out)
                for _ in gp:
                    pass

    nc.compile()

    # All activation funcs used (Exp/Ln/Square/Identity) live in the
    # natural_log_exp_and_others table; bacc's greedy insertion alternates
    # between two tables instead.  Point every load at the shared table and
    # drop the now-redundant reloads (they carry no sync info).
    NLE = 6  # index of natural_log_exp_and_others in act_info.json
    for b in (nc.m.functions[0].blocks if unify_tables else []):
        insts = b.instructions
        kept = []
        seen = False
        for inst in insts:
            if isinstance(inst, mybir.InstLoadActFuncSet):
                assert not (inst.has_wait() or inst.has_update())
                if seen:
                    continue
                inst.act_func_set_id = NLE
                seen = True
            kept.append(inst)
        b.instructions = kept
    return nc


# --------------------------------------------------------------------------
# host wrapper
# --------------------------------------------------------------------------

def _prep_in_maps(x, qkv_w, qkv_b, q_norm_w, k_norm_w, proj_w, T, H, W):
    x2 = np.ascontiguousarray(x[0].T).astype(np.float32) * SX  # (2048, 4096)
    xh, xl = _split8(x2)
    cos_ext, sin_ext = _rope_ext_tables(T, H, W)
    wq = q_norm_w.astype(np.float32)
    wk_ = k_norm_w.astype(np.float32)
    # rope tables absorb the 1/SQK descale of qb16 (and the norm weight)
    cosq = (cos_ext * wq[:, None] / SQK).astype(F16)
    sinq = (sin_ext * _pair_swap(wq)[:, None] / SQK).astype(F16)
    cosk = (cos_ext * wk_[:, None] / SQK).astype(F16)
    sink = (sin_ext * _pair_swap(wk_)[:, None] / SQK).astype(F16)

    perm = np.zeros((P, P), np.float32)
    idx = np.arange(P)
    perm[idx ^ 1, idx] = 1.0   # psum_qs[m, n] = q[pair(m), n]
    perm = perm.astype(F16)
    ident = np.eye(P, dtype=np.float32).astype(F16)

    in_maps = []
    for c in range(NCORES):
        h0 = HPC * c
        blocks = []
        bias_blocks = []
        bias2_blocks = []
        for h in (h0, h0 + 1):
            for t3 in range(3):
                r0 = t3 * DIM + h * HD
                ws = SWV if t3 == 2 else SWQK
                bs = SX * ws
                blocks.append(qkv_w[r0:r0 + HD].astype(np.float32) * ws)
                bias_blocks.append(qkv_b[r0:r0 + HD].astype(np.float32) * bs)
                bias2_blocks.append(qkv_b[r0:r0 + HD].astype(np.float32))
        wt_c = np.concatenate(blocks, axis=0)               # (768, 2048)
        wt_c = np.ascontiguousarray(wt_c.T)                 # (2048, 768)
        wh_c, wl_c = _split8(wt_c, lo_dt=F8L)
        bq_c = np.stack(bias_blocks, axis=1).astype(np.float32)   # (128, 6)
        bq2_c = np.stack(bias2_blocks, axis=1).astype(np.float32)
        # wp: (128 d, 2 heads, 2048), scaled by SWP, fp8 hi/lo
        wp_c = np.stack(
            [np.ascontiguousarray(
                proj_w[:, (h0 + h) * HD:(h0 + h + 1) * HD].T).astype(
                    np.float32) * SWP
             for h in range(HPC)], axis=1)                  # (128, 2, 2048)
        wph_c, wpl_c = _split8(wp_c)
        in_maps.append({
            "xh": xh, "xl": xl, "wh": wh_c, "wl": wl_c,
            "bqkv": bq_c, "bqkv2": bq2_c,
            "wph": wph_c, "wpl": wpl_c,
            "cosq": cosq, "sinq": sinq, "cosk": cosk, "sink": sink,
            "ones16": np.ones((P, 1), F16),
            "ones8": np.ones((P, 2, 16), F8),
            "perm": perm, "ident": ident,
            "epsc": np.full((2, 1), RMS_EPS, np.float32),
            "neg2": np.full((P, 1), -2.0, np.float32),
        })
    return in_maps


def kernel(x, qkv_w, qkv_b, q_norm_w, k_norm_w, proj_w, proj_b, T, H, W):
    x = np.asarray(x)
    T, H, W = int(T), int(H), int(W)
    assert x.shape == (1, N, DIM) and T * H * W == N

    if "nc" not in _CACHE:
        _CACHE["nc"] = _build_nc()
    nc = _CACHE["nc"]

    in_maps = _prep_in_maps(
        x, np.asarray(qkv_w), np.asarray(qkv_b), np.asarray(q_norm_w),
        np.asarray(k_norm_w), np.asarray(proj_w), T, H, W)

    res = run_bass_kernel_spmd(nc, in_maps, core_ids=list(range(NCORES)))
    out = np.zeros((N, DIM), np.float64)
    for c in range(NCORES):
        out += res.results[c]["out_p"].astype(np.float64)
    out = (out / OUT_DIV).astype(np.float32) \
        + np.asarray(proj_b, np.float32)[None, :]
    return out[None].astype(x.dtype)
